# revision 20
# baseline (speedup 1.0000x reference)
"""Trainium2 Bass kernel for nn_Decoder_10239202034039.

8-way head/tensor-parallel decoder layer:
  - per-head MLP stacks + attention sharded over the head axis H=8 (1 head/core)
  - AllGather for the shared qkv activations (each head consumes the full qkv)
  - concat-layer GEMM computed as per-head partial sums -> ReduceScatter over
    the sequence axis S, so the second concat GEMM + layernorm run S-sharded
  - tiny AllReduce for layernorm statistics (ln over S with S column-sharded)
  - AllToAll to switch from S-shard to D-shard for the output head
All weights are pre-transposed on the host into the exact [K, M]/[K, N]
layouts the TensorEngine consumes (out = kxm.T @ kxn).
"""

import os
import sys

for _p in ("/opt/trn_rl_repo", "/root/.axon_site/_ro/trn_rl_repo"):
    if os.path.isdir(_p) and _p not in sys.path:
        sys.path.insert(0, _p)

import numpy as np

import concourse.bacc as bacc
import concourse.bass as bass
import concourse.mybir as mybir
import concourse.tile as tile
from concourse.bass_utils import run_bass_kernel_spmd
from concourse.kernels.tile_matmul import matmul_tile_kernel
from concourse.masks import make_identity

H, D, S, MID, S_OUT = 8, 1024, 1024, 2048, 512
NCORES = 8
SSH = S // NCORES  # 128 sequence columns per core
DSH = D // NCORES  # 128 d rows per core (output head shard)
EPS = 1e-6
FP = mybir.dt.float32
RG = [list(range(NCORES))]

Act = mybir.ActivationFunctionType
Alu = mybir.AluOpType


def _decl_inputs(nc):
    d = {}

    def inp(name, shape):
        d[name] = nc.declare_dram_parameter(name, list(shape), FP, isOutput=False)

    inp("inp_sh", (D, SSH))
    inp("w1f", (D, 3 * D))
    inp("w2q", (D, D))
    inp("ekT", (S, D))
    inp("evT", (S, D))
    for b in (1, 2):
        inp(f"m{b}_W1T", (3, S, MID))
        inp(f"m{b}_W2T", (3, MID, S))
        inp(f"cw1T_{b}", (D, MID))
        inp(f"cw2T_{b}", (MID, D))
    inp("l1w1T", (S, MID))
    inp("l1w2T", (MID, S_OUT))
    inp("l2w1T", (S_OUT, MID))
    inp("l2w2T", (MID, S_OUT))
    inp("gam_b", (128, SSH))
    inp("bet_b", (128, SSH))
    d["out_sh"] = nc.declare_dram_parameter("out_sh", [DSH, S_OUT], FP, isOutput=True)
    return d


def _build_program():
    # KSTAGE: bisection aid — 1: qkv+AG; 2: +block1->RS1; 3: +LN1/AG2;
    # 4: +block2; 9: full (default)
    KSTAGE = int(os.environ.get("KSTAGE", "9"))
    nc = bacc.Bacc(None, num_devices=NCORES)
    P = _decl_inputs(nc)

    with tile.TileContext(nc) as tc:
        _emit_body(nc, tc, P, KSTAGE)
    nc.compile()
    return nc


def _emit_body(nc, tc, P, KSTAGE):
    if True:
        with tc.tile_pool(name="dram", bufs=1, space="DRAM") as dram, \
             tc.tile_pool(name="kxm_pool", bufs=5) as kxm_pool, \
             tc.tile_pool(name="kxn_pool", bufs=5) as kxn_pool, \
             tc.tile_pool(name="const", bufs=1) as const, \
             tc.tile_pool(name="aux", bufs=3) as aux, \
             tc.tile_pool(name="lnp", bufs=1) as lnp, \
             tc.tile_pool(name="tr_ps", bufs=2, space="PSUM") as tr_ps:

            ident = const.tile([128, 128], FP, name="ident")
            make_identity(nc, ident)
            eps_t = const.tile([128, 1], FP, name="eps_t")
            nc.vector.memset(eps_t[:], float(EPS))
            gam_t = const.tile([128, SSH], FP, name="gam_t")
            bet_t = const.tile([128, SSH], FP, name="bet_t")
            nc.sync.dma_start(gam_t[:], P["gam_b"][:])
            nc.sync.dma_start(bet_t[:], P["bet_b"][:])

            def gemm(kxm, kxn, out, relu=False, evict=None):
                matmul_tile_kernel(
                    tc, kxm, kxn, out,
                    use_relu=relu,
                    psum_evict_fn=evict,
                    kxm_pool=kxm_pool,
                    kxn_pool=kxn_pool,
                )

            def transpose(src, dst, rows, cols, relu=False):
                # src [rows, cols] (DRAM) -> dst [cols, rows] (DRAM), via PE
                for ci in range(cols // 128):
                    for ri in range(rows // 128):
                        t_in = aux.tile([128, 128], FP, tag="tr_in")
                        nc.sync.dma_start(
                            t_in[:], src[ri * 128:(ri + 1) * 128, ci * 128:(ci + 1) * 128])
                        ps = tr_ps.tile([128, 128], FP, tag="tr_psum")
                        nc.tensor.transpose(ps[:], t_in[:], ident[:])
                        t_out = aux.tile([128, 128], FP, tag="tr_out")
                        if relu:
                            nc.scalar.activation(t_out[:], ps[:], Act.Relu)
                        else:
                            nc.scalar.activation(t_out[:], ps[:], Act.Copy)
                        nc.sync.dma_start(
                            dst[ci * 128:(ci + 1) * 128, ri * 128:(ri + 1) * 128], t_out[:])

            def collective(kind, op, in_ap, out_ap):
                nc.gpsimd.collective_compute(
                    kind, op, replica_groups=RG,
                    ins=[in_ap.opt()], outs=[out_ap.opt()])

            def layernorm(b, y2t, h_out, transpose_relu_to=None):
                """x = inp_sh + y2t ([D, SSH], d on partitions in 8 tiles);
                ln over the (distributed) S axis via stats AllReduce.
                h_out [D, SSH]; optionally also emit relu(h).T tiles into
                transpose_relu_to [S, DSH-worth...] rows t*128 (a2a input)."""
                stats_p = dram.tile([2, D], FP, name=f"stats_p{b}")
                stats_f = dram.tile([2, D], FP, name=f"stats_f{b}")
                xts = []
                for t in range(8):
                    sl = slice(t * 128, (t + 1) * 128)
                    xt = lnp.tile([128, SSH], FP, name=f"x_{b}_{t}")
                    rt = lnp.tile([128, SSH], FP, tag="ln_res")
                    nc.sync.dma_start(xt[:], y2t[sl, :])
                    nc.sync.dma_start(rt[:], P["inp_sh"][sl, :])
                    nc.vector.tensor_add(xt[:], xt[:], rt[:])
                    sm = lnp.tile([128, 1], FP, tag="ln_sum")
                    nc.vector.reduce_sum(sm[:], xt[:], axis=mybir.AxisListType.X)
                    sq = lnp.tile([128, SSH], FP, tag="ln_sq")
                    ssq = lnp.tile([128, 1], FP, tag="ln_ssq")
                    nc.scalar.activation(sq[:], xt[:], Act.Square, accum_out=ssq[:])
                    nc.sync.dma_start(stats_p[0, sl], sm[:])
                    nc.sync.dma_start(stats_p[1, sl], ssq[:])
                    xts.append(xt)
                collective("AllReduce", Alu.add, stats_p[:], stats_f[:])
                for t in range(8):
                    sl = slice(t * 128, (t + 1) * 128)
                    xt = xts[t]
                    sm = lnp.tile([128, 1], FP, tag="ln_sum2")
                    ssq = lnp.tile([128, 1], FP, tag="ln_ssq2")
                    nc.sync.dma_start(sm[:], stats_f[0, sl])
                    nc.sync.dma_start(ssq[:], stats_f[1, sl])
                    mu = lnp.tile([128, 1], FP, tag="ln_mu")
                    nc.scalar.mul(mu[:], sm[:], 1.0 / S)
                    ex2 = lnp.tile([128, 1], FP, tag="ln_ex2")
                    nc.scalar.mul(ex2[:], ssq[:], 1.0 / S)
                    musq = lnp.tile([128, 1], FP, tag="ln_musq")
                    nc.vector.tensor_mul(musq[:], mu[:], mu[:])
                    var = lnp.tile([128, 1], FP, tag="ln_var")
                    nc.vector.tensor_sub(var[:], ex2[:], musq[:])
                    std = lnp.tile([128, 1], FP, tag="ln_std")
                    nc.scalar.activation(std[:], var[:], Act.Sqrt, bias=eps_t[:])
                    rstd = lnp.tile([128, 1], FP, tag="ln_rstd")
                    nc.vector.reciprocal(rstd[:], std[:])
                    hn = lnp.tile([128, SSH], FP, tag="ln_hn")
                    nc.vector.tensor_scalar(
                        hn[:], xt[:], mu[:], rstd[:], op0=Alu.subtract, op1=Alu.mult)
                    nc.vector.tensor_mul(hn[:], hn[:], gam_t[:])
                    nc.vector.tensor_add(hn[:], hn[:], bet_t[:])
                    nc.sync.dma_start(h_out[sl, :], hn[:])
                    if transpose_relu_to is not None:
                        ps = tr_ps.tile([128, 128], FP, tag="tr_psum")
                        nc.tensor.transpose(ps[:], hn[:], ident[:])
                        t_out = aux.tile([128, 128], FP, tag="tr_out")
                        nc.scalar.activation(t_out[:], ps[:], Act.Relu)
                        nc.sync.dma_start(
                            transpose_relu_to[t * 128:(t + 1) * 128, :], t_out[:])

            def finish_early(src2d):
                # copy a window into out_sh to keep truncated stages live
                w = min(src2d.shape[1], S_OUT)
                t = aux.tile([128, S_OUT], FP, tag="fin")
                nc.sync.dma_start(t[:, :w], src2d[:, :w])
                nc.sync.dma_start(P["out_sh"][:, :w], t[:, :w])

            # ---- stage 0: qkv1T (relu'd), sharded compute + AllGather ----
            qkp = dram.tile([SSH, 3 * D], FP, name="qkp")
            qkf = dram.tile([S, 3 * D], FP, name="qkf", addr_space="Shared")
            gemm(P["inp_sh"][:], P["w1f"][:], qkp[:], relu=True)
            collective("AllGather", Alu.bypass, qkp[:], qkf[:])
            if KSTAGE == 1:
                finish_early(qkf[:DSH, :S_OUT])
                return

            h1 = dram.tile([D, SSH], FP, name="h1")
            q2f = dram.tile([S, D], FP, name="q2f", addr_space="Shared")
            a2a_i = dram.tile([S, DSH], FP, name="a2a_i")
            a2a_o = dram.tile([S, DSH], FP, name="a2a_o")

            zt = {b: [dram.tile([MID, D], FP, name=f"zt{b}_{q}") for q in range(3)]
                  for b in (1, 2)}

            for b in (1, 2):
                W1T, W2T = P[f"m{b}_W1T"], P[f"m{b}_W2T"]
                if b == 1:
                    QT = [qkf[:, q * D:(q + 1) * D] for q in range(3)]
                else:
                    QT = [q2f[:], P["ekT"][:], P["evT"][:]]

                sq_ = dram.tile([D, S], FP, name=f"sq{b}")
                sk_ = dram.tile([D, S], FP, name=f"sk{b}")
                vT = dram.tile([S, D], FP, name=f"vT{b}")
                bm = dram.tile([S, S], FP, name=f"bm{b}")
                att = dram.tile([D, S], FP, name=f"att{b}")
                y1p = dram.tile([S, MID], FP, name=f"y1p{b}")
                y1s = dram.tile([SSH, MID], FP, name=f"y1s{b}")
                y1t = dram.tile([MID, SSH], FP, name=f"y1t{b}")
                y2t = dram.tile([D, SSH], FP, name=f"y2t{b}")

                qs = range(3) if b == 1 else (0,)  # b=2: q=1,2 precomputed below
                for q in qs:
                    gemm(W1T[q], QT[q], zt[b][q][:], relu=True)  # relu(zT) [MID, D]
                gemm(zt[b][0][:], W2T[0], sq_[:])              # q  [D, S]
                gemm(zt[b][1][:], W2T[1], sk_[:])              # k  [D, S]
                gemm(W2T[2], zt[b][2][:], vT[:])               # vT [S, D]
                gemm(sk_[:], sq_[:], bm[:])                    # b = k.T q  [S, S]
                gemm(vT[:], bm[:], att[:], relu=True)          # relu(att) [D, S]
                gemm(att[:], P[f"cw1T_{b}"][:], y1p[:])        # y1 partial [S, MID]
                collective("ReduceScatter", Alu.add, y1p[:], y1s[:])
                if KSTAGE == 2 and b == 1:
                    finish_early(y1s[:, :S_OUT])
                    return
                if b == 1 and KSTAGE >= 4:
                    # independent block-2 encoder-side MLP inputs: overlap with RS
                    gemm(P["m2_W1T"][1], P["ekT"][:], zt[2][1][:], relu=True)
                    gemm(P["m2_W1T"][2], P["evT"][:], zt[2][2][:], relu=True)
                transpose(y1s[:], y1t[:], SSH, MID, relu=True)  # relu(y1).T [MID, SSH]
                gemm(P[f"cw2T_{b}"][:], y1t[:], y2t[:])         # m[:, s_c]  [D, SSH]

                if b == 1:
                    layernorm(b, y2t[:], h1[:])
                    q2p = dram.tile([SSH, D], FP, name="q2p")
                    gemm(h1[:], P["w2q"][:], q2p[:], relu=True)  # relu(q2T)[s_c] [SSH, D]
                    collective("AllGather", Alu.bypass, q2p[:], q2f[:])
                    if KSTAGE == 3:
                        finish_early(q2f[:DSH, :S_OUT])
                        return
                else:
                    h2 = dram.tile([D, SSH], FP, name="h2")
                    layernorm(b, y2t[:], h2[:], transpose_relu_to=a2a_i[:])

            if KSTAGE == 4:
                finish_early(a2a_i[:DSH, :])
                return

            # ---- output head on D-shard via AllToAll ----
            collective("AllToAll", Alu.bypass, a2a_i[:], a2a_o[:])
            if KSTAGE == 5:
                finish_early(a2a_o[:DSH, :])
                return
            x1t = dram.tile([MID, DSH], FP, name="x1t")
            x2 = dram.tile([DSH, S_OUT], FP, name="x2")
            x2t = dram.tile([S_OUT, DSH], FP, name="x2t")
            y3 = dram.tile([DSH, MID], FP, name="y3")
            y3t = dram.tile([MID, DSH], FP, name="y3t")
            gemm(P["l1w1T"][:], a2a_o[:], x1t[:], relu=True)
            gemm(x1t[:], P["l1w2T"][:], x2[:], relu=True)
            transpose(x2[:], x2t[:], DSH, S_OUT)
            gemm(x2t[:], P["l2w1T"][:], y3[:], relu=True)
            transpose(y3[:], y3t[:], DSH, MID)
            if KSTAGE == 6:
                finish_early(y3t[:DSH, :])
                return

            if KSTAGE == 7:
                gemm(y3t[:], P["l2w2T"][:], P["out_sh"][:])
                return

            # Tanh/Exp act tables fault on this HW setup; the pre-tanh values
            # here satisfy |x| <~ 0.25, where the Pade [3/2] form
            # tanh(x) ~= x*(27 + x^2) / (27 + 9*x^2) is accurate to ~1e-8
            # (still 7e-4 at |x|=1).
            y4 = dram.tile([DSH, S_OUT], FP, name="y4")
            gemm(y3t[:], P["l2w2T"][:], y4[:])
            t_in = aux.tile([128, S_OUT], FP, tag="tanh_in")
            t_sq = aux.tile([128, S_OUT], FP, tag="tanh_sq")
            t_num = aux.tile([128, S_OUT], FP, tag="tanh_num")
            t_den = aux.tile([128, S_OUT], FP, tag="tanh_den")
            t_out = aux.tile([128, S_OUT], FP, tag="tanh_out")
            nc.sync.dma_start(t_in[:], y4[:])
            nc.vector.tensor_mul(t_sq[:], t_in[:], t_in[:])
            nc.vector.tensor_scalar_add(t_num[:], t_sq[:], 27.0)
            nc.vector.tensor_mul(t_num[:], t_num[:], t_in[:])
            nc.vector.tensor_scalar(
                t_den[:], t_sq[:], 9.0, 27.0, op0=Alu.mult, op1=Alu.add)
            nc.vector.reciprocal(t_den[:], t_den[:])
            nc.vector.tensor_mul(t_out[:], t_num[:], t_den[:])
            nc.sync.dma_start(P["out_sh"][:], t_out[:])

    nc.compile()
    return nc


_NC_CACHE = None


def _get_program():
    global _NC_CACHE
    if _NC_CACHE is None:
        _NC_CACHE = _build_program()
    return _NC_CACHE


def _host_prep(inputs):
    """Build the 8 per-core input maps from the full problem inputs."""
    f32 = lambda a: np.ascontiguousarray(np.asarray(a, dtype=np.float32))
    inp = f32(inputs["inp"])
    enc_k, enc_v = f32(inputs["enc_k"]), f32(inputs["enc_v"])
    w_qkv1, w_qkv2 = f32(inputs["w_qkv1"]), f32(inputs["w_qkv2"])
    mh_W1 = {1: f32(inputs["mh1_W1"]), 2: f32(inputs["mh2_W1"])}
    mh_W2 = {1: f32(inputs["mh1_W2"]), 2: f32(inputs["mh2_W2"])}
    c_w1 = {1: f32(inputs["c1_w1"]), 2: f32(inputs["c2_w1"])}
    c_w2 = {1: f32(inputs["c1_w2"]), 2: f32(inputs["c2_w2"])}
    gamma, beta = f32(inputs["gamma"]), f32(inputs["beta"])

    w1f = np.ascontiguousarray(
        np.concatenate([w_qkv1[q].T for q in range(3)], axis=1))  # [D, 3D]
    w2q = np.ascontiguousarray(w_qkv2[0].T)
    ekT = np.ascontiguousarray(np.maximum(enc_k, 0.0).T)  # relu'd [S, D]
    evT = np.ascontiguousarray(np.maximum(enc_v, 0.0).T)
    shared = {
        "w1f": w1f, "w2q": w2q, "ekT": ekT, "evT": evT,
        "l1w1T": np.ascontiguousarray(f32(inputs["l1_w1"]).T),
        "l1w2T": np.ascontiguousarray(f32(inputs["l1_w2"]).T),
        "l2w1T": np.ascontiguousarray(f32(inputs["l2_w1"]).T),
        "l2w2T": np.ascontiguousarray(f32(inputs["l2_w2"]).T),
    }
    for b in (1, 2):
        shared[f"cw2T_{b}"] = np.ascontiguousarray(c_w2[b].T)

    in_maps = []
    for c in range(NCORES):
        sl = slice(c * SSH, (c + 1) * SSH)
        m = dict(shared)
        m["inp_sh"] = np.ascontiguousarray(inp[:, sl])
        m["gam_b"] = np.ascontiguousarray(np.tile(gamma[sl][None, :], (128, 1)))
        m["bet_b"] = np.ascontiguousarray(np.tile(beta[sl][None, :], (128, 1)))
        for b in (1, 2):
            m[f"m{b}_W1T"] = np.ascontiguousarray(mh_W1[b][c].transpose(0, 2, 1))
            m[f"m{b}_W2T"] = np.ascontiguousarray(mh_W2[b][c].transpose(0, 2, 1))
            m[f"cw1T_{b}"] = np.ascontiguousarray(c_w1[b][:, c * D:(c + 1) * D].T)
        in_maps.append(m)
    return in_maps


def kernel(**inputs) -> np.ndarray:
    nc = _get_program()
    in_maps = _host_prep(inputs)
    res = run_bass_kernel_spmd(nc, in_maps, list(range(NCORES)))
    out = np.concatenate([res.results[c]["out_sh"] for c in range(NCORES)], axis=0)
    return out.astype(np.float32)


# revision 23
# speedup vs baseline: 2.1702x; 2.1702x over previous
"""Trainium2 Bass kernel for nn_Decoder_10239202034039.

8-way head/tensor-parallel decoder layer:
  - per-head MLP stacks + attention sharded over the head axis H=8 (1 head/core)
  - AllGather for the shared qkv activations (each head consumes the full qkv)
  - concat-layer GEMM computed as per-head partial sums -> ReduceScatter over
    the sequence axis S, so the second concat GEMM + layernorm run S-sharded
  - tiny AllReduce for layernorm statistics (ln over S with S column-sharded)
  - AllToAll to switch from S-shard to D-shard for the output head
All weights are pre-transposed on the host into the exact [K, M]/[K, N]
layouts the TensorEngine consumes (out = kxm.T @ kxn).
"""

import os
import sys

for _p in ("/opt/trn_rl_repo", "/root/.axon_site/_ro/trn_rl_repo"):
    if os.path.isdir(_p) and _p not in sys.path:
        sys.path.insert(0, _p)

import numpy as np

import concourse.bacc as bacc
import concourse.bass as bass
import concourse.mybir as mybir
import concourse.tile as tile
from concourse.bass_utils import run_bass_kernel_spmd
from concourse.kernels.tile_matmul import matmul_tile_kernel
from concourse.masks import make_identity

H, D, S, MID, S_OUT = 8, 1024, 1024, 2048, 512
NCORES = 8
SSH = S // NCORES  # 128 sequence columns per core
DSH = D // NCORES  # 128 d rows per core (output head shard)
EPS = 1e-6
FP = mybir.dt.float32
# float32r: same bits as fp32, single-pass reduced-precision matmul at 4x
# the fp32 rate (fp32 runs 2 half-speed passes). Big GEMM chain only.
FPR = mybir.dt.float32 if os.environ.get("KF32R", "1") == "0" else mybir.dt.float32r
RG = [list(range(NCORES))]

Act = mybir.ActivationFunctionType
Alu = mybir.AluOpType


def _decl_inputs(nc):
    d = {}

    def inp(name, shape, dt=FP):
        d[name] = nc.declare_dram_parameter(name, list(shape), dt, isOutput=False)

    inp("inp_sh", (D, SSH), FPR)
    inp("inp_r", (D, SSH))
    inp("w1f", (D, 3 * D), FPR)
    inp("w2q", (D, D), FPR)
    inp("ekT", (S, D), FPR)
    inp("evT", (S, D), FPR)
    for b in (1, 2):
        inp(f"m{b}_W1T", (3, S, MID), FPR)
        inp(f"m{b}_W2T", (3, MID, S), FPR)
        inp(f"cw1T_{b}", (D, MID), FPR)
        inp(f"cw2T_{b}", (MID, D), FPR)
    inp("l1w1T", (S, MID), FPR)
    inp("l1w2T", (MID, S_OUT), FPR)
    inp("l2w1T", (S_OUT, MID), FPR)
    inp("l2w2T", (MID, S_OUT), FPR)
    inp("gam_b", (128, SSH))
    inp("bet_b", (128, SSH))
    d["out_sh"] = nc.declare_dram_parameter("out_sh", [DSH, S_OUT], FP, isOutput=True)
    return d


def _build_program():
    # KSTAGE: bisection aid — 1: qkv+AG; 2: +block1->RS1; 3: +LN1/AG2;
    # 4: +block2; 9: full (default)
    KSTAGE = int(os.environ.get("KSTAGE", "9"))
    nc = bacc.Bacc(None, num_devices=NCORES)
    P = _decl_inputs(nc)

    with tile.TileContext(nc) as tc:
        _emit_body(nc, tc, P, KSTAGE)
    nc.compile()
    return nc


def _emit_body(nc, tc, P, KSTAGE):
    if True:
        with tc.tile_pool(name="dram", bufs=1, space="DRAM") as dram, \
             tc.tile_pool(name="kxm_pool", bufs=5) as kxm_pool, \
             tc.tile_pool(name="kxn_pool", bufs=5) as kxn_pool, \
             tc.tile_pool(name="const", bufs=1) as const, \
             tc.tile_pool(name="aux", bufs=3) as aux, \
             tc.tile_pool(name="lnp", bufs=1) as lnp, \
             tc.tile_pool(name="tr_ps", bufs=2, space="PSUM") as tr_ps:

            ident = const.tile([128, 128], FP, name="ident")
            make_identity(nc, ident)
            eps_t = const.tile([128, 1], FP, name="eps_t")
            nc.vector.memset(eps_t[:], float(EPS))
            gam_t = const.tile([128, SSH], FP, name="gam_t")
            bet_t = const.tile([128, SSH], FP, name="bet_t")
            nc.sync.dma_start(gam_t[:], P["gam_b"][:])
            nc.sync.dma_start(bet_t[:], P["bet_b"][:])

            def gemm(kxm, kxn, out, relu=False, evict=None):
                matmul_tile_kernel(
                    tc, kxm, kxn, out,
                    use_relu=relu,
                    psum_evict_fn=evict,
                    kxm_pool=kxm_pool,
                    kxn_pool=kxn_pool,
                )

            def transpose(src, dst, rows, cols, relu=False):
                # src [rows, cols] (DRAM) -> dst [cols, rows] (DRAM), via PE
                odt = dst.tensor.dtype
                for ci in range(cols // 128):
                    for ri in range(rows // 128):
                        t_in = aux.tile([128, 128], FP, tag="tr_in")
                        nc.sync.dma_start(
                            t_in[:], src[ri * 128:(ri + 1) * 128, ci * 128:(ci + 1) * 128])
                        ps = tr_ps.tile([128, 128], FP, tag="tr_psum")
                        nc.tensor.transpose(ps[:], t_in[:], ident[:])
                        t_out = aux.tile([128, 128], odt, tag=f"tr_out_{odt}")
                        if relu:
                            nc.scalar.activation(t_out[:], ps[:], Act.Relu)
                        else:
                            nc.scalar.activation(t_out[:], ps[:], Act.Copy)
                        nc.sync.dma_start(
                            dst[ci * 128:(ci + 1) * 128, ri * 128:(ri + 1) * 128], t_out[:])

            def collective(kind, op, in_ap, out_ap):
                nc.gpsimd.collective_compute(
                    kind, op, replica_groups=RG,
                    ins=[in_ap.opt()], outs=[out_ap.opt()])

            def layernorm(b, y2t, h_out, transpose_relu_to=None):
                """x = inp_sh + y2t ([D, SSH], d on partitions in 8 tiles);
                ln over the (distributed) S axis via stats AllReduce.
                h_out [D, SSH]; optionally also emit relu(h).T tiles into
                transpose_relu_to [S, DSH-worth...] rows t*128 (a2a input)."""
                stats_p = dram.tile([2, D], FP, name=f"stats_p{b}")
                stats_f = dram.tile([2, D], FP, name=f"stats_f{b}")
                xts = []
                for t in range(8):
                    sl = slice(t * 128, (t + 1) * 128)
                    xt = lnp.tile([128, SSH], FP, name=f"x_{b}_{t}")
                    rt = lnp.tile([128, SSH], FP, tag="ln_res")
                    nc.sync.dma_start(xt[:], y2t[sl, :])
                    nc.sync.dma_start(rt[:], P["inp_r"][sl, :])
                    nc.vector.tensor_add(xt[:], xt[:], rt[:])
                    sm = lnp.tile([128, 1], FP, tag="ln_sum")
                    nc.vector.reduce_sum(sm[:], xt[:], axis=mybir.AxisListType.X)
                    sq = lnp.tile([128, SSH], FP, tag="ln_sq")
                    ssq = lnp.tile([128, 1], FP, tag="ln_ssq")
                    nc.scalar.activation(sq[:], xt[:], Act.Square, accum_out=ssq[:])
                    nc.sync.dma_start(stats_p[0, sl], sm[:])
                    nc.sync.dma_start(stats_p[1, sl], ssq[:])
                    xts.append(xt)
                collective("AllReduce", Alu.add, stats_p[:], stats_f[:])
                for t in range(8):
                    sl = slice(t * 128, (t + 1) * 128)
                    xt = xts[t]
                    sm = lnp.tile([128, 1], FP, tag="ln_sum2")
                    ssq = lnp.tile([128, 1], FP, tag="ln_ssq2")
                    nc.sync.dma_start(sm[:], stats_f[0, sl])
                    nc.sync.dma_start(ssq[:], stats_f[1, sl])
                    mu = lnp.tile([128, 1], FP, tag="ln_mu")
                    nc.scalar.mul(mu[:], sm[:], 1.0 / S)
                    ex2 = lnp.tile([128, 1], FP, tag="ln_ex2")
                    nc.scalar.mul(ex2[:], ssq[:], 1.0 / S)
                    musq = lnp.tile([128, 1], FP, tag="ln_musq")
                    nc.vector.tensor_mul(musq[:], mu[:], mu[:])
                    var = lnp.tile([128, 1], FP, tag="ln_var")
                    nc.vector.tensor_sub(var[:], ex2[:], musq[:])
                    std = lnp.tile([128, 1], FP, tag="ln_std")
                    nc.scalar.activation(std[:], var[:], Act.Sqrt, bias=eps_t[:])
                    rstd = lnp.tile([128, 1], FP, tag="ln_rstd")
                    nc.vector.reciprocal(rstd[:], std[:])
                    hn = lnp.tile([128, SSH], FP, tag="ln_hn")
                    nc.vector.tensor_scalar(
                        hn[:], xt[:], mu[:], rstd[:], op0=Alu.subtract, op1=Alu.mult)
                    nc.vector.tensor_mul(hn[:], hn[:], gam_t[:])
                    hdt = h_out.tensor.dtype
                    hn_o = lnp.tile([128, SSH], hdt, tag="ln_hn_o")
                    nc.vector.tensor_add(hn_o[:], hn[:], bet_t[:])
                    nc.sync.dma_start(h_out[sl, :], hn_o[:])
                    if transpose_relu_to is not None:
                        adt = transpose_relu_to.tensor.dtype
                        ps = tr_ps.tile([128, 128], FP, tag="tr_psum")
                        nc.tensor.transpose(ps[:], hn_o[:], ident[:])
                        t_out = aux.tile([128, 128], adt, tag=f"tr_out_{adt}")
                        nc.scalar.activation(t_out[:], ps[:], Act.Relu)
                        nc.sync.dma_start(
                            transpose_relu_to[t * 128:(t + 1) * 128, :], t_out[:])

            def finish_early(src2d):
                # copy a window into out_sh to keep truncated stages live
                w = min(src2d.shape[1], S_OUT)
                t = lnp.tile([128, S_OUT], FP, tag="fin")
                nc.gpsimd.dma_start(t[:, :w], src2d[:, :w])
                nc.gpsimd.dma_start(P["out_sh"][:, :w], t[:, :w])

            # ---- stage 0: qkv1T (relu'd), sharded compute + AllGather ----
            qkp = dram.tile([SSH, 3 * D], FPR, name="qkp")
            qkf = dram.tile([S, 3 * D], FPR, name="qkf", addr_space="Shared")
            gemm(P["inp_sh"][:], P["w1f"][:], qkp[:], relu=True)
            collective("AllGather", Alu.bypass, qkp[:], qkf[:])
            if KSTAGE == 1:
                finish_early(qkf[:DSH, :S_OUT])
                return

            h1 = dram.tile([D, SSH], FPR, name="h1")
            q2f = dram.tile([S, D], FPR, name="q2f", addr_space="Shared")
            a2a_i = dram.tile([S, DSH], FPR, name="a2a_i")
            a2a_o = dram.tile([S, DSH], FPR, name="a2a_o")

            zt = {b: [dram.tile([MID, D], FPR, name=f"zt{b}_{q}") for q in range(3)]
                  for b in (1, 2)}

            for b in (1, 2):
                W1T, W2T = P[f"m{b}_W1T"], P[f"m{b}_W2T"]
                if b == 1:
                    QT = [qkf[:, q * D:(q + 1) * D] for q in range(3)]
                else:
                    QT = [q2f[:], P["ekT"][:], P["evT"][:]]

                sq_ = dram.tile([D, S], FPR, name=f"sq{b}")
                sk_ = dram.tile([D, S], FPR, name=f"sk{b}")
                vT = dram.tile([S, D], FPR, name=f"vT{b}")
                bm = dram.tile([S, S], FPR, name=f"bm{b}")
                att = dram.tile([D, S], FPR, name=f"att{b}")
                y1p = dram.tile([S, MID], FP, name=f"y1p{b}")
                y1s = dram.tile([SSH, MID], FP, name=f"y1s{b}")
                y1t = dram.tile([MID, SSH], FPR, name=f"y1t{b}")
                y2t = dram.tile([D, SSH], FP, name=f"y2t{b}")

                qs = range(3) if b == 1 else (0,)  # b=2: q=1,2 precomputed below
                for q in qs:
                    gemm(W1T[q], QT[q], zt[b][q][:], relu=True)  # relu(zT) [MID, D]
                gemm(zt[b][0][:], W2T[0], sq_[:])              # q  [D, S]
                gemm(zt[b][1][:], W2T[1], sk_[:])              # k  [D, S]
                gemm(W2T[2], zt[b][2][:], vT[:])               # vT [S, D]
                gemm(sk_[:], sq_[:], bm[:])                    # b = k.T q  [S, S]
                gemm(vT[:], bm[:], att[:], relu=True)          # relu(att) [D, S]
                gemm(att[:], P[f"cw1T_{b}"][:], y1p[:])        # y1 partial [S, MID]
                collective("ReduceScatter", Alu.add, y1p[:], y1s[:])
                if KSTAGE == 2 and b == 1:
                    finish_early(y1s[:, :S_OUT])
                    return
                if b == 1 and KSTAGE >= 4:
                    # independent block-2 encoder-side MLP inputs: overlap with RS
                    gemm(P["m2_W1T"][1], P["ekT"][:], zt[2][1][:], relu=True)
                    gemm(P["m2_W1T"][2], P["evT"][:], zt[2][2][:], relu=True)
                transpose(y1s[:], y1t[:], SSH, MID, relu=True)  # relu(y1).T [MID, SSH]
                gemm(P[f"cw2T_{b}"][:], y1t[:], y2t[:])         # m[:, s_c]  [D, SSH]

                if b == 1:
                    layernorm(b, y2t[:], h1[:])
                    q2p = dram.tile([SSH, D], FPR, name="q2p")
                    gemm(h1[:], P["w2q"][:], q2p[:], relu=True)  # relu(q2T)[s_c] [SSH, D]
                    collective("AllGather", Alu.bypass, q2p[:], q2f[:])
                    if KSTAGE == 3:
                        finish_early(q2f[:DSH, :S_OUT])
                        return
                else:
                    h2 = dram.tile([D, SSH], FP, name="h2")
                    layernorm(b, y2t[:], h2[:], transpose_relu_to=a2a_i[:])

            if KSTAGE == 4:
                finish_early(a2a_i[:DSH, :])
                return

            # ---- output head on D-shard via AllToAll ----
            collective("AllToAll", Alu.bypass, a2a_i[:], a2a_o[:])
            if KSTAGE == 5:
                finish_early(a2a_o[:DSH, :])
                return
            x1t = dram.tile([MID, DSH], FPR, name="x1t")
            x2 = dram.tile([DSH, S_OUT], FP, name="x2")
            x2t = dram.tile([S_OUT, DSH], FPR, name="x2t")
            y3 = dram.tile([DSH, MID], FP, name="y3")
            y3t = dram.tile([MID, DSH], FPR, name="y3t")
            gemm(P["l1w1T"][:], a2a_o[:], x1t[:], relu=True)
            gemm(x1t[:], P["l1w2T"][:], x2[:], relu=True)
            transpose(x2[:], x2t[:], DSH, S_OUT)
            gemm(x2t[:], P["l2w1T"][:], y3[:], relu=True)
            transpose(y3[:], y3t[:], DSH, MID)
            if KSTAGE == 6:
                finish_early(y3t[:DSH, :])
                return

            if KSTAGE == 7:
                gemm(y3t[:], P["l2w2T"][:], P["out_sh"][:])
                return

            # Tanh/Exp act tables fault on this HW setup; the pre-tanh values
            # here satisfy |x| <~ 0.25, where the Pade [3/2] form
            # tanh(x) ~= x*(27 + x^2) / (27 + 9*x^2) is accurate to ~1e-8
            # (still 7e-4 at |x|=1).
            y4 = dram.tile([DSH, S_OUT], FP, name="y4")
            gemm(y3t[:], P["l2w2T"][:], y4[:])
            t_in = lnp.tile([128, S_OUT], FP, tag="tanh_in")
            t_sq = lnp.tile([128, S_OUT], FP, tag="tanh_sq")
            t_num = lnp.tile([128, S_OUT], FP, tag="tanh_num")
            t_den = lnp.tile([128, S_OUT], FP, tag="tanh_den")
            t_out = lnp.tile([128, S_OUT], FP, tag="tanh_out")
            nc.sync.dma_start(t_in[:], y4[:])
            nc.vector.tensor_mul(t_sq[:], t_in[:], t_in[:])
            nc.vector.tensor_scalar_add(t_num[:], t_sq[:], 27.0)
            nc.vector.tensor_mul(t_num[:], t_num[:], t_in[:])
            nc.vector.tensor_scalar(
                t_den[:], t_sq[:], 9.0, 27.0, op0=Alu.mult, op1=Alu.add)
            nc.vector.reciprocal(t_den[:], t_den[:])
            nc.vector.tensor_mul(t_out[:], t_num[:], t_den[:])
            nc.sync.dma_start(P["out_sh"][:], t_out[:])

    nc.compile()
    return nc


_NC_CACHE = None


def _get_program():
    global _NC_CACHE
    if _NC_CACHE is None:
        _NC_CACHE = _build_program()
    return _NC_CACHE


def _host_prep(inputs):
    """Build the 8 per-core input maps from the full problem inputs."""
    f32 = lambda a: np.ascontiguousarray(np.asarray(a, dtype=np.float32))
    inp = f32(inputs["inp"])
    enc_k, enc_v = f32(inputs["enc_k"]), f32(inputs["enc_v"])
    w_qkv1, w_qkv2 = f32(inputs["w_qkv1"]), f32(inputs["w_qkv2"])
    mh_W1 = {1: f32(inputs["mh1_W1"]), 2: f32(inputs["mh2_W1"])}
    mh_W2 = {1: f32(inputs["mh1_W2"]), 2: f32(inputs["mh2_W2"])}
    c_w1 = {1: f32(inputs["c1_w1"]), 2: f32(inputs["c2_w1"])}
    c_w2 = {1: f32(inputs["c1_w2"]), 2: f32(inputs["c2_w2"])}
    gamma, beta = f32(inputs["gamma"]), f32(inputs["beta"])

    w1f = np.ascontiguousarray(
        np.concatenate([w_qkv1[q].T for q in range(3)], axis=1))  # [D, 3D]
    w2q = np.ascontiguousarray(w_qkv2[0].T)
    ekT = np.ascontiguousarray(np.maximum(enc_k, 0.0).T)  # relu'd [S, D]
    evT = np.ascontiguousarray(np.maximum(enc_v, 0.0).T)
    shared = {
        "w1f": w1f, "w2q": w2q, "ekT": ekT, "evT": evT,
        "l1w1T": np.ascontiguousarray(f32(inputs["l1_w1"]).T),
        "l1w2T": np.ascontiguousarray(f32(inputs["l1_w2"]).T),
        "l2w1T": np.ascontiguousarray(f32(inputs["l2_w1"]).T),
        "l2w2T": np.ascontiguousarray(f32(inputs["l2_w2"]).T),
    }
    for b in (1, 2):
        shared[f"cw2T_{b}"] = np.ascontiguousarray(c_w2[b].T)

    in_maps = []
    for c in range(NCORES):
        sl = slice(c * SSH, (c + 1) * SSH)
        m = dict(shared)
        m["inp_sh"] = np.ascontiguousarray(inp[:, sl])
        m["inp_r"] = m["inp_sh"]
        m["gam_b"] = np.ascontiguousarray(np.tile(gamma[sl][None, :], (128, 1)))
        m["bet_b"] = np.ascontiguousarray(np.tile(beta[sl][None, :], (128, 1)))
        for b in (1, 2):
            m[f"m{b}_W1T"] = np.ascontiguousarray(mh_W1[b][c].transpose(0, 2, 1))
            m[f"m{b}_W2T"] = np.ascontiguousarray(mh_W2[b][c].transpose(0, 2, 1))
            m[f"cw1T_{b}"] = np.ascontiguousarray(c_w1[b][:, c * D:(c + 1) * D].T)
        in_maps.append(m)
    return in_maps


def kernel(**inputs) -> np.ndarray:
    nc = _get_program()
    in_maps = _host_prep(inputs)
    res = run_bass_kernel_spmd(nc, in_maps, list(range(NCORES)))
    out = np.concatenate([res.results[c]["out_sh"] for c in range(NCORES)], axis=0)
    return out.astype(np.float32)
